# revision 2
# baseline (speedup 1.0000x reference)
"""DSQG attention kernel for 8 Trainium2 NeuronCores.

Sharding: core c = (b, half) with b = c//2 in 0..3, half = c%2.
Each core handles batch b and heads [half*8, half*8+8):
  - qkv+gate projection for its 512 channels (bias folded in via ones-column)
  - 44-tap dyadic attention (24 distinct lags; duplicate-offset pos_bias terms
    folded into per-lag multiplicative weights W[L,h] = sum_dup exp(pos_bias))
  - gated partial output projection through its 512 rows of w_out
Host sums the two half partials per batch and adds b_out.

Shifted k/v windows are fetched per (block, lag) from a zero-padded DRAM
buffer: DRAM rows have no partition-alignment constraint, unlike SBUF reads
(start partition must be 0/32/64/96).
"""
import sys

sys.path.insert(0, "/opt/trn_rl_repo")

import numpy as np
import ml_dtypes

N_SCALES = 11
N_TAPS = 4
OFFSETS = [(1 << j) * tau for j in range(N_SCALES) for tau in range(N_TAPS)]
LAGS = sorted(set(OFFSETS))  # 24 distinct lags
NL = len(LAGS)
B, N, D, H = 4, 4096, 1024, 16
HD = 64
HH = 8  # heads per core
CW = HH * HD  # 512 channels per core
KE = 1152  # padded contraction: 1024 x-cols + 1 ones-col + 127 zero pad
NBLK = N // 128  # 32
PAD = 3072  # zero rows above qkvg for causal lookback
SCALE = HD ** -0.5

# lags needing a DMA window fetch (walrus requires identical start partitions
# on all TensorTensor operands, so only 128-multiples can read SBUF directly)
SMALL = [1, 2, 3, 4, 6, 8, 12, 16, 24, 32, 48, 64, 96, 192]

_CACHE = {}


def _pieces(L):
    """(out_lo, out_hi, tile_delta, src_lo) for lags that are 128-multiples."""
    qd, r = divmod(L, 128)
    assert r == 0, L
    return [(0, 128, qd, 0)]


def _build_program():
    import concourse.bacc as bacc
    import concourse.mybir as mybir
    import concourse.tile as tile
    from concourse.kernels.tile_matmul import matmul_tile_kernel

    bf16 = mybir.dt.bfloat16
    f32 = mybir.dt.float32

    nc = bacc.Bacc("TRN2", target_bir_lowering=False, debug=False, num_devices=8)
    xb = nc.dram_tensor("xb", [N, KE], bf16, kind="ExternalInput").ap()
    wqg = nc.dram_tensor("wqg", [KE, 4 * CW], bf16, kind="ExternalInput").ap()
    wo = nc.dram_tensor("wo", [CW, D], bf16, kind="ExternalInput").ap()
    wrep = nc.dram_tensor("wrep", [128, HH * NL], f32, kind="ExternalInput").ap()
    y = nc.dram_tensor("y", [N, D], f32, kind="ExternalOutput").ap()

    with tile.TileContext(nc) as tc:
        with (
            tc.tile_pool(name="dram", bufs=1, space="DRAM") as dram,
            tc.tile_pool(name="kvpool", bufs=1) as kvpool,
            tc.tile_pool(name="work", bufs=2) as work,
            tc.tile_pool(name="winp", bufs=16) as winp,
            tc.tile_pool(name="prodp", bufs=1) as prodp,
            tc.tile_pool(name="constp", bufs=1) as constp,
        ):
            qkvg = dram.tile([PAD + N, 4 * CW], bf16)
            gtil = dram.tile([N, CW], bf16)

            # zero the pad rows (read back by the small-lag window DMAs)
            ztile = constp.tile([128, 2 * CW], bf16, tag="ztile")
            nc.vector.memset(ztile[:], 0.0)
            for t in range(PAD // 128):
                nc.sync.dma_start(out=qkvg[t * 128:(t + 1) * 128, CW:3 * CW], in_=ztile[:])

            # Phase 1: [q|k|v|gate] = x_ext @ w_ext   (biases via ones column)
            matmul_tile_kernel(tc, xb, wqg, qkvg[PAD:, :], transpose_kxm=True)
            tc.strict_bb_all_engine_barrier()

            wr = constp.tile([128, HH * NL], f32, tag="wr")
            nc.sync.dma_start(out=wr[:], in_=wrep[:])

            # k, v resident in SBUF for the quadrant-alignable lags
            kb = kvpool.tile([128, NBLK * CW], bf16, tag="kb")
            vb = kvpool.tile([128, NBLK * CW], bf16, tag="vb")
            for a in range(NBLK):
                rws = slice(PAD + a * 128, PAD + (a + 1) * 128)
                nc.sync.dma_start(out=kb[:, a * CW:(a + 1) * CW], in_=qkvg[rws, CW:2 * CW])
                nc.sync.dma_start(out=vb[:, a * CW:(a + 1) * CW], in_=qkvg[rws, 2 * CW:3 * CW])

            GRP = 1  # blocks per small-lag window fetch group

            def fetch_small_windows(g):
                """One DMA per small lag: k and v windows for GRP blocks.

                Returns {L: tile [128, GRP*2*CW] viewed (p, a_local, kv, c)}."""
                wins = {}
                base = PAD + g * GRP * 128
                for i, L in enumerate(SMALL):
                    win = winp.tile([128, GRP * 2 * CW], bf16, tag="win")
                    eng = nc.sync if (i % 2 == 0) else nc.scalar
                    src = qkvg[base - L:base - L + GRP * 128, CW:3 * CW]
                    eng.dma_start(out=win[:], in_=src)
                    wins[L] = win
                return wins

            def tap_muls(a0, wins, prod, lhs_of, resident, kv):
                """Emit prod[:, j*CW:(j+1)*CW] = lhs ⊙ window_L for every lag.

                lhs_of(j): 3D [128, HH, HD] left operand for lag slot j.
                resident: SBUF tensor (kb/vb) for 128-multiple lags;
                kv: 0 for k windows, 1 for v windows (small lags)."""
                def r3(ap):
                    return ap.rearrange("p (h d) -> p h d", h=HH)

                al = a0 % GRP
                for j, L in enumerate(LAGS):
                    dst = prod[:, j * CW:(j + 1) * CW]
                    if L in SMALL:
                        w0 = (al * 2 + kv) * CW
                        nc.vector.tensor_mul(
                            r3(dst), lhs_of(j), r3(wins[L][:, w0:w0 + CW]))
                        continue
                    a = a0 - L // 128
                    if a < 0:
                        nc.vector.memset(dst[:, :], 0.0)
                    else:
                        nc.vector.tensor_mul(
                            r3(dst), lhs_of(j),
                            r3(resident[:, a * CW:(a + 1) * CW]))

            # Phase 2: attention middle, per 128-row block
            wins = None
            for a0 in range(NBLK):
                if a0 % GRP == 0:
                    wins = fetch_small_windows(a0 // GRP)
                n0 = a0 * 128
                rows = slice(PAD + n0, PAD + n0 + 128)
                qb = work.tile([128, CW], bf16, tag="qb")
                nc.sync.dma_start(out=qb[:], in_=qkvg[rows, 0:CW])
                prod = prodp.tile([128, NL * CW], bf16, tag="prod")

                # scores products: prod[:, j*CW:(j+1)*CW] = q * k[n-L]
                tap_muls(a0, wins, prod,
                         lambda j: qb[:].rearrange("p (h d) -> p h d", h=HH),
                         kb, 0)
                # reduce over d -> scores [128, (h, j)]
                scores = work.tile([128, HH * NL], f32, tag="scores")
                nc.vector.tensor_reduce(
                    out=scores[:].rearrange("p (h l) -> p l h", h=HH, l=NL).unsqueeze(3),
                    in_=prod[:].rearrange("p (l h d) -> p l h d", l=NL, h=HH),
                    axis=mybir.AxisListType.X,
                    op=mybir.AluOpType.add,
                )
                # softmax over lags with folded pos-bias weights, unnormalized
                ew = work.tile([128, HH * NL], f32, tag="ew")
                nc.scalar.activation(ew[:], scores[:], mybir.ActivationFunctionType.Exp, scale=SCALE)
                ew2 = work.tile([128, HH * NL], f32, tag="ew2")
                nc.vector.tensor_mul(ew2[:], ew[:], wr[:])
                z = work.tile([128, HH], f32, tag="z")
                nc.vector.tensor_reduce(
                    out=z[:].unsqueeze(2),
                    in_=ew2[:].rearrange("p (h l) -> p h l", h=HH),
                    axis=mybir.AxisListType.X,
                    op=mybir.AluOpType.add,
                )
                rz = work.tile([128, HH], f32, tag="rz")
                nc.vector.reciprocal(rz[:], z[:])
                attn = work.tile([128, HH * NL], bf16, tag="attn")
                nc.vector.tensor_mul(
                    attn[:].rearrange("p (h l) -> p h l", h=HH),
                    ew2[:].rearrange("p (h l) -> p h l", h=HH),
                    rz[:].unsqueeze(2).broadcast_to([128, HH, NL]),
                )
                # weighted taps: prod[:, j] = attn[:, :, j] * v[n-L]
                attn3 = attn[:].rearrange("p (h l) -> p h l", h=HH)

                def attn_lhs(j):
                    return attn3[:, :, j:j + 1].broadcast_to([128, HH, HD])

                tap_muls(a0, wins, prod, attn_lhs, vb, 1)
                accv = work.tile([128, CW], f32, tag="accv")
                nc.vector.tensor_reduce(
                    out=accv[:].rearrange("p (h d) -> p h d", h=HH).unsqueeze(3),
                    in_=prod[:].rearrange("p (l h d) -> p h d l", l=NL, h=HH),
                    axis=mybir.AxisListType.X,
                    op=mybir.AluOpType.add,
                )
                # gate and gathered*gate
                gt = work.tile([128, CW], bf16, tag="gt")
                nc.sync.dma_start(out=gt[:], in_=qkvg[rows, 3 * CW:4 * CW])
                gsig = work.tile([128, CW], f32, tag="gsig")
                nc.scalar.activation(gsig[:], gt[:], mybir.ActivationFunctionType.Sigmoid)
                gtl = work.tile([128, CW], bf16, tag="gtl")
                nc.vector.tensor_mul(gtl[:], accv[:], gsig[:])
                nc.sync.dma_start(out=gtil[n0:n0 + 128, :], in_=gtl[:])

            # Phase 3: y_partial = (gathered*gate) @ w_out[half rows]
            tc.strict_bb_all_engine_barrier()
            matmul_tile_kernel(tc, gtil[:], wo, y, transpose_kxm=True)

    nc.compile()
    return nc


def _get_program():
    if "nc" not in _CACHE:
        _CACHE["nc"] = _build_program()
    return _CACHE["nc"]


def _core_inputs(x, w_qkv, b_qkv, w_gate, b_gate, w_out, pos_bias, b, half):
    bf = ml_dtypes.bfloat16
    cs = slice(half * CW, (half + 1) * CW)

    xb = np.zeros((N, KE), dtype=bf)
    xb[:, :D] = x[b].astype(bf)
    xb[:, D] = np.asarray(1.0, dtype=bf)

    wqg = np.zeros((KE, 4 * CW), dtype=np.float32)
    wqg[:D, 0:CW] = w_qkv[:, cs]
    wqg[:D, CW:2 * CW] = w_qkv[:, D + cs.start:D + cs.stop]
    wqg[:D, 2 * CW:3 * CW] = w_qkv[:, 2 * D + cs.start:2 * D + cs.stop]
    wqg[:D, 3 * CW:4 * CW] = w_gate[:, cs]
    wqg[D, 0:CW] = b_qkv[cs]
    wqg[D, CW:2 * CW] = b_qkv[D + cs.start:D + cs.stop]
    wqg[D, 2 * CW:3 * CW] = b_qkv[2 * D + cs.start:2 * D + cs.stop]
    wqg[D, 3 * CW:4 * CW] = b_gate[cs]

    # lag weights: W[h, j] = sum over duplicate offsets of exp(pos_bias[i, h])
    wl = np.zeros((HH, NL), dtype=np.float32)
    for i, off in enumerate(OFFSETS):
        j = LAGS.index(off)
        wl[:, j] += np.exp(pos_bias[i, half * HH:(half + 1) * HH].astype(np.float64)).astype(np.float32)
    wrep = np.broadcast_to(wl.reshape(1, HH * NL), (128, HH * NL)).copy()

    return {
        "xb": xb,
        "wqg": wqg.astype(bf),
        "wo": w_out[cs, :].astype(bf),
        "wrep": wrep,
    }


def _in_maps(inputs):
    return [
        _core_inputs(inputs["x"], inputs["w_qkv"], inputs["b_qkv"], inputs["w_gate"],
                     inputs["b_gate"], inputs["w_out"], inputs["pos_bias"], c // 2, c % 2)
        for c in range(8)
    ]


def kernel(x, w_qkv, b_qkv, w_out, b_out, w_gate, b_gate, pos_bias):
    from concourse.bass_utils import run_bass_kernel_spmd

    x = np.asarray(x, dtype=np.float32)
    w_qkv = np.asarray(w_qkv, dtype=np.float32)
    b_qkv = np.asarray(b_qkv, dtype=np.float32)
    w_out = np.asarray(w_out, dtype=np.float32)
    b_out = np.asarray(b_out, dtype=np.float32)
    w_gate = np.asarray(w_gate, dtype=np.float32)
    b_gate = np.asarray(b_gate, dtype=np.float32)
    pos_bias = np.asarray(pos_bias, dtype=np.float32)

    nc = _get_program()
    in_maps = [
        _core_inputs(x, w_qkv, b_qkv, w_gate, b_gate, w_out, pos_bias, c // 2, c % 2)
        for c in range(8)
    ]
    res = run_bass_kernel_spmd(nc, in_maps, core_ids=list(range(8)))
    out = np.empty((B, N, D), dtype=np.float32)
    for b in range(B):
        out[b] = res.results[2 * b]["y"] + res.results[2 * b + 1]["y"] + b_out[None, :]
    return out



# revision 6
# speedup vs baseline: 2.1883x; 2.1883x over previous
"""DSQG attention kernel for 8 Trainium2 NeuronCores — v2 (transposed layout).

Sharding: core c = (b, half): batch b = c//2, heads [half*8, half*8+8).

All on-chip tensors live in TRANSPOSED layout [channels on partitions, seq on
free]: causal lag shifts become free-dim slices (no partition-alignment issues,
no DMA window fetches). Per core:

  ph1: qT/kT/vT/gT [512ch, 4096] = W^T-stationary matmuls over xT (bias via
       ones-row). kT/vT carry 512 zero-pad columns for causal lookback.
  A:   per (ptile t, lag L): P = qT_t * kT_t[:, n-L]  (DVE, bf16 2x)
       scores via selector-matmul  sc[32s+h, n] += E_t^T @ P  (TensorE reduce
       over d; 24 lags in 6 PSUM "quad" banks, 32-aligned slots).
  B:   esc = exp(SCALE*sc + ln wl)  (ScalarE, dup-offset pos_bias weights wl
       folded via per-partition bias); Z via selector-matmul; rz = 1/Z.
  C:   abc = S_{t,s}^T @ esc  (TensorE row-broadcast) -> ScalarE copy to bf16
       -> prod = abc * vT_t[:, n-L]  (DVE 2x) -> out_t += I^T @ prod
       (TensorE identity-matmul accumulation in PSUM).
  m:   mT_t = sigmoid(gT_t) * (R4_t^T @ rz) * out_t   (normalize at the end)
  ph3: yT[Do, n] = wo^T-stationary matmuls over mT; host transposes, sums the
       two half partials and adds b_out.
"""
import sys

sys.path.insert(0, "/opt/trn_rl_repo")

import numpy as np
import ml_dtypes

N_SCALES = 11
N_TAPS = 4
OFFSETS = [(1 << j) * tau for j in range(N_SCALES) for tau in range(N_TAPS)]
LAGS = sorted(set(OFFSETS))  # 24 distinct lags
NL = len(LAGS)
NQ = NL // 4  # 6 quad groups of 4 lags
B, N, D, H = 4, 4096, 1024, 16
HD = 64
HH = 8           # heads per core
CW = HH * HD     # 512 channels per core
NT = 4           # channel ptiles per core (128 ch each, 2 heads per ptile)
KE = 1152        # padded contraction: 1024 x-rows + ones-row + 127 zero pad
KP = KE // 128   # 9
PADN = 512       # zero columns in front of kT/vT for causal lookback
NTile = 512      # seq columns per ph2 tile
NNT = N // NTile  # 8
SCALE = HD ** -0.5

_CACHE = {}

# const packing offsets within cb [128, CB_W] bf16
CB_E4 = 0                  # 4 x [128, 32] selectors E_t
CB_E2 = CB_E4 + 4 * 32     # [128, 8] Z selector
CB_S2 = CB_E2 + 8          # 16 x [128, 128] broadcast selectors S_{t,s}
CB_I = CB_S2 + 16 * 128    # [128, 128] identity
CB_W = CB_I + 128


def _live(L, n0):
    # window [n0-L, n0-L+NTile) intersects valid v/k rows (reads stay inside
    # the PADN zero region otherwise)
    return L < n0 + NTile


def _build_program():
    import concourse.bacc as bacc
    import concourse.mybir as mybir
    import concourse.tile as tile

    bf16 = mybir.dt.bfloat16
    f32 = mybir.dt.float32
    Act = mybir.ActivationFunctionType

    nc = bacc.Bacc("TRN2", target_bir_lowering=False, debug=False, num_devices=8)
    xT = nc.dram_tensor("xT", [KE, N], bf16, kind="ExternalInput").ap()
    wqg = nc.dram_tensor("wqg", [KE, 4 * CW], bf16, kind="ExternalInput").ap()
    wo = nc.dram_tensor("wo", [CW, D], bf16, kind="ExternalInput").ap()
    cb = nc.dram_tensor("cb", [128, CB_W], bf16, kind="ExternalInput").ap()
    cf = nc.dram_tensor("cf", [128, NQ], f32, kind="ExternalInput").ap()
    r4 = nc.dram_tensor("r4", [HH, 4 * 128], f32, kind="ExternalInput").ap()
    yT = nc.dram_tensor("yT", [D, N], f32, kind="ExternalOutput").ap()

    with tile.TileContext(nc) as tc:
        with (
            tc.tile_pool(name="constp", bufs=1) as constp,
            tc.tile_pool(name="pers", bufs=1) as pers,
        ):
            cbt = constp.tile([128, CB_W], bf16, tag="cb")
            nc.sync.dma_start(out=cbt[:], in_=cb[:, :])
            cft = constp.tile([128, NQ], f32, tag="cf")
            nc.sync.dma_start(out=cft[:], in_=cf[:, :])
            r4t = constp.tile([HH, 4 * 128], f32, tag="r4")
            nc.sync.dma_start(out=r4t[:], in_=r4[:, :])

            qT = [pers.tile([128, N], bf16, tag=f"qT{t}", name=f"qT{t}") for t in range(NT)]
            kT = [pers.tile([128, PADN + N], bf16, tag=f"kT{t}", name=f"kT{t}") for t in range(NT)]
            vT = [pers.tile([128, PADN + N], bf16, tag=f"vT{t}", name=f"vT{t}") for t in range(NT)]
            gT = [pers.tile([128, N], bf16, tag=f"gT{t}", name=f"gT{t}") for t in range(NT)]
            mT = qT  # qT columns are dead after stage A reads them; reuse as m
            for t in range(NT):
                nc.vector.memset(kT[t][:, 0:PADN], 0.0)
                nc.vector.memset(vT[t][:, 0:PADN], 0.0)

            # ---- Phase 1: transposed qkv+gate projection ----
            with (
                tc.tile_pool(name="xp", bufs=1) as xp,
                tc.tile_pool(name="wp", bufs=4) as wp,
                tc.tile_pool(name="pp1", bufs=8, space="PSUM") as pp1,
            ):
                for half in range(2):
                    xts = []
                    for kp in range(KP):
                        xt = xp.tile([128, N // 2], bf16, tag=f"x{kp}")
                        nc.sync.dma_start(
                            out=xt[:],
                            in_=xT[kp * 128:(kp + 1) * 128,
                                   half * (N // 2):(half + 1) * (N // 2)])
                        xts.append(xt)
                    for ch in range(16):
                        ps = [pp1.tile([128, NTile], f32, tag="p1", name="p1") for _ in range(4)]
                        for kp in range(KP):
                            wt = wp.tile([128, 128], bf16, tag="wt")
                            nc.sync.dma_start(
                                out=wt[:],
                                in_=wqg[kp * 128:(kp + 1) * 128,
                                        ch * 128:(ch + 1) * 128])
                            for ns in range(4):
                                nc.tensor.matmul(
                                    out=ps[ns][:],
                                    lhsT=wt[:],
                                    rhs=xts[kp][:, ns * NTile:(ns + 1) * NTile],
                                    start=(kp == 0), stop=(kp == KP - 1))
                        grp, t = divmod(ch, NT)
                        dst = [qT, kT, vT, gT][grp][t]
                        off = (PADN if grp in (1, 2) else 0) + half * (N // 2)
                        for ns in range(4):
                            nc.scalar.activation(
                                dst[:, off + ns * NTile: off + (ns + 1) * NTile],
                                ps[ns][:], Act.Copy)

            # ---- Phase 2: attention middle ----
            with (
                tc.tile_pool(name="work", bufs=3) as work,
                tc.tile_pool(name="escp", bufs=8) as escp,
                tc.tile_pool(name="rzp", bufs=2) as rzp,
                tc.tile_pool(name="scp", bufs=2, space="PSUM") as scp,
                tc.tile_pool(name="zp", bufs=1, space="PSUM") as zp,
                tc.tile_pool(name="abcp", bufs=2, space="PSUM") as abcp,
                tc.tile_pool(name="outp", bufs=2, space="PSUM") as outp,
            ):
                for nt in range(NNT):
                    n0 = nt * NTile
                    escs = []
                    zt = zp.tile([HH, NTile], f32, tag="z")
                    for q in range(NQ):
                        sct = scp.tile([128, NTile], f32, tag="sc")
                        for s in range(4):
                            L = LAGS[q * 4 + s]
                            if not _live(L, n0):
                                nc.vector.memset(sct[32 * s:32 * s + 32, :], 0.0)
                                continue
                            for t in range(NT):
                                pt = work.tile([128, NTile], bf16, tag="P")
                                nc.vector.tensor_mul(
                                    pt[:],
                                    qT[t][:, n0:n0 + NTile],
                                    kT[t][:, PADN + n0 - L: PADN + n0 - L + NTile])
                                nc.tensor.matmul(
                                    out=sct[32 * s:32 * s + 32, :],
                                    lhsT=cbt[:, CB_E4 + 32 * t: CB_E4 + 32 * (t + 1)],
                                    rhs=pt[:],
                                    start=(t == 0), stop=(t == NT - 1),
                                    tile_position=(0, 32 * s))
                        esc = escp.tile([128, NTile], bf16, tag="esc")
                        nc.scalar.activation(esc[:], sct[:], Act.Exp,
                                             scale=SCALE, bias=cft[:, q:q + 1])
                        escs.append(esc)
                        nc.tensor.matmul(
                            out=zt[:],
                            lhsT=cbt[:, CB_E2: CB_E2 + HH],
                            rhs=esc[:],
                            start=(q == 0), stop=(q == NQ - 1))
                    rz = rzp.tile([HH, NTile], f32, tag="rz")
                    nc.vector.reciprocal(rz[:], zt[:])

                    live = [(q, s) for q in range(NQ) for s in range(4)
                            if _live(LAGS[q * 4 + s], n0)]
                    for t in range(NT):
                        outps = outp.tile([128, NTile], f32, tag="out")
                        for i, (q, s) in enumerate(live):
                            L = LAGS[q * 4 + s]
                            abc = abcp.tile([128, NTile], f32, tag="abc")
                            nc.tensor.matmul(
                                out=abc[:],
                                lhsT=cbt[:, CB_S2 + (t * 4 + s) * 128:
                                         CB_S2 + (t * 4 + s + 1) * 128],
                                rhs=escs[q][:],
                                start=True, stop=True)
                            absb = work.tile([128, NTile], bf16, tag="absb")
                            nc.scalar.activation(absb[:], abc[:], Act.Copy)
                            prod = work.tile([128, NTile], bf16, tag="prod")
                            nc.vector.tensor_mul(
                                prod[:], absb[:],
                                vT[t][:, PADN + n0 - L: PADN + n0 - L + NTile])
                            nc.tensor.matmul(
                                out=outps[:],
                                lhsT=cbt[:, CB_I: CB_I + 128],
                                rhs=prod[:],
                                start=(i == 0), stop=(i == len(live) - 1))
                        # finalize: m = sigmoid(g) * (1/Z bcast) * gathered
                        gs = work.tile([128, NTile], bf16, tag="gs")
                        nc.scalar.activation(gs[:], gT[t][:, n0:n0 + NTile],
                                             Act.Sigmoid)
                        rzb = abcp.tile([128, NTile], f32, tag="abc")
                        nc.tensor.matmul(
                            out=rzb[:],
                            lhsT=r4t[:, t * 128:(t + 1) * 128],
                            rhs=rz[:],
                            start=True, stop=True)
                        tmp = work.tile([128, NTile], bf16, tag="tmp")
                        nc.vector.tensor_mul(tmp[:], gs[:], rzb[:])
                        nc.vector.tensor_mul(
                            mT[t][:, n0:n0 + NTile], tmp[:], outps[:])

            # ---- Phase 3: transposed out projection ----
            with (
                tc.tile_pool(name="wp3", bufs=4) as wp3,
                tc.tile_pool(name="ys", bufs=4) as ys,
                tc.tile_pool(name="pp3", bufs=8, space="PSUM") as pp3,
            ):
                for do in range(D // 128):
                    pss = [pp3.tile([128, NTile], f32, tag="p3", name="p3") for _ in range(NNT)]
                    for ct in range(NT):
                        wt3 = wp3.tile([128, 128], bf16, tag="wt3")
                        nc.sync.dma_start(
                            out=wt3[:],
                            in_=wo[ct * 128:(ct + 1) * 128, do * 128:(do + 1) * 128])
                        for ns in range(NNT):
                            nc.tensor.matmul(
                                out=pss[ns][:],
                                lhsT=wt3[:],
                                rhs=mT[ct][:, ns * NTile:(ns + 1) * NTile],
                                start=(ct == 0), stop=(ct == NT - 1))
                    for ns in range(NNT):
                        yst = ys.tile([128, NTile], f32, tag="yst")
                        nc.scalar.activation(yst[:], pss[ns][:], Act.Copy)
                        nc.sync.dma_start(
                            out=yT[do * 128:(do + 1) * 128,
                                   ns * NTile:(ns + 1) * NTile],
                            in_=yst[:])

    nc.compile()
    return nc


def _get_program():
    if "nc" not in _CACHE:
        _CACHE["nc"] = _build_program()
    return _CACHE["nc"]


def _core_inputs(x, w_qkv, b_qkv, w_gate, b_gate, w_out, pos_bias, b, half):
    bf = ml_dtypes.bfloat16
    cs = slice(half * CW, (half + 1) * CW)

    xTa = np.zeros((KE, N), dtype=bf)
    xTa[:D, :] = x[b].T.astype(bf)
    xTa[D, :] = np.asarray(1.0, dtype=bf)

    wqg = np.zeros((KE, 4 * CW), dtype=np.float32)
    wqg[:D, 0:CW] = w_qkv[:, cs]
    wqg[:D, CW:2 * CW] = w_qkv[:, D + cs.start:D + cs.stop]
    wqg[:D, 2 * CW:3 * CW] = w_qkv[:, 2 * D + cs.start:2 * D + cs.stop]
    wqg[:D, 3 * CW:4 * CW] = w_gate[:, cs]
    wqg[D, 0:CW] = b_qkv[cs]
    wqg[D, CW:2 * CW] = b_qkv[D + cs.start:D + cs.stop]
    wqg[D, 2 * CW:3 * CW] = b_qkv[2 * D + cs.start:2 * D + cs.stop]
    wqg[D, 3 * CW:4 * CW] = b_gate[cs]

    # wl[h, j] = sum over duplicate offsets of exp(pos_bias[i, h])  (local heads)
    wl = np.zeros((HH, NL), dtype=np.float64)
    for i, off in enumerate(OFFSETS):
        j = LAGS.index(off)
        wl[:, j] += np.exp(pos_bias[i, half * HH:(half + 1) * HH].astype(np.float64))
    lnwl = np.log(wl)  # [HH, NL]

    # bf16 consts: selectors
    cba = np.zeros((128, CB_W), dtype=bf)
    p = np.arange(128)
    for t in range(NT):
        for j in range(2):  # local heads 2t, 2t+1
            col = CB_E4 + 32 * t + 2 * t + j
            cba[:, col] = (p // 64 == j).astype(bf)
    for h in range(HH):
        cba[:, CB_E2 + h] = (p % 32 == h).astype(bf)
    for t in range(NT):
        for s in range(4):
            blk = np.zeros((128, 128), dtype=bf)
            for pd in range(128):
                blk[32 * s + 2 * t + pd // 64, pd] = 1
            cba[:, CB_S2 + (t * 4 + s) * 128: CB_S2 + (t * 4 + s + 1) * 128] = blk
    cba[:, CB_I: CB_I + 128] = np.eye(128, dtype=np.float32).astype(bf)

    # f32 consts: lnwl at rows 32*s + h, col q
    cfa = np.zeros((128, NQ), dtype=np.float32)
    for q in range(NQ):
        for s in range(4):
            for h in range(HH):
                cfa[32 * s + h, q] = lnwl[h, q * 4 + s]

    r4a = np.zeros((HH, 4 * 128), dtype=np.float32)
    for t in range(NT):
        for pd in range(128):
            r4a[2 * t + pd // 64, t * 128 + pd] = 1.0

    return {
        "xT": xTa,
        "wqg": wqg.astype(bf),
        "wo": w_out[cs, :].astype(bf),
        "cb": cba,
        "cf": cfa,
        "r4": r4a,
    }


def _in_maps(inputs):
    return [
        _core_inputs(inputs["x"], inputs["w_qkv"], inputs["b_qkv"], inputs["w_gate"],
                     inputs["b_gate"], inputs["w_out"], inputs["pos_bias"], c // 2, c % 2)
        for c in range(8)
    ]


def kernel(x, w_qkv, b_qkv, w_out, b_out, w_gate, b_gate, pos_bias):
    from concourse.bass_utils import run_bass_kernel_spmd

    x = np.asarray(x, dtype=np.float32)
    w_qkv = np.asarray(w_qkv, dtype=np.float32)
    b_qkv = np.asarray(b_qkv, dtype=np.float32)
    w_out = np.asarray(w_out, dtype=np.float32)
    b_out = np.asarray(b_out, dtype=np.float32)
    w_gate = np.asarray(w_gate, dtype=np.float32)
    b_gate = np.asarray(b_gate, dtype=np.float32)
    pos_bias = np.asarray(pos_bias, dtype=np.float32)

    nc = _get_program()
    in_maps = _in_maps({
        "x": x, "w_qkv": w_qkv, "b_qkv": b_qkv, "w_gate": w_gate,
        "b_gate": b_gate, "w_out": w_out, "pos_bias": pos_bias,
    })
    res = run_bass_kernel_spmd(nc, in_maps, core_ids=list(range(8)))
    out = np.empty((B, N, D), dtype=np.float32)
    for b in range(B):
        out[b] = (res.results[2 * b]["yT"] + res.results[2 * b + 1]["yT"]).T \
            + b_out[None, :]
    return out


# revision 7
# speedup vs baseline: 2.2889x; 1.0460x over previous
"""DSQG attention kernel for 8 Trainium2 NeuronCores — v2 (transposed layout).

Sharding: core c = (b, half): batch b = c//2, heads [half*8, half*8+8).

All on-chip tensors live in TRANSPOSED layout [channels on partitions, seq on
free]: causal lag shifts become free-dim slices (no partition-alignment issues,
no DMA window fetches). Per core:

  ph1: qT/kT/vT/gT [512ch, 4096] = W^T-stationary matmuls over xT (bias via
       ones-row). kT/vT carry 512 zero-pad columns for causal lookback.
  A:   per (ptile t, lag L): P = qT_t * kT_t[:, n-L]  (DVE, bf16 2x)
       scores via selector-matmul  sc[32s+h, n] += E_t^T @ P  (TensorE reduce
       over d; 24 lags in 6 PSUM "quad" banks, 32-aligned slots).
  B:   esc = exp(SCALE*sc + ln wl)  (ScalarE, dup-offset pos_bias weights wl
       folded via per-partition bias); Z via selector-matmul; rz = 1/Z.
  C:   abc = S_{t,s}^T @ esc  (TensorE row-broadcast) -> ScalarE copy to bf16
       -> prod = abc * vT_t[:, n-L]  (DVE 2x) -> out_t += I^T @ prod
       (TensorE identity-matmul accumulation in PSUM).
  m:   mT_t = sigmoid(gT_t) * (R4_t^T @ rz) * out_t   (normalize at the end)
  ph3: yT[Do, n] = wo^T-stationary matmuls over mT; host transposes, sums the
       two half partials and adds b_out.
"""
import sys

sys.path.insert(0, "/opt/trn_rl_repo")

import numpy as np
import ml_dtypes

N_SCALES = 11
N_TAPS = 4
OFFSETS = [(1 << j) * tau for j in range(N_SCALES) for tau in range(N_TAPS)]
LAGS = sorted(set(OFFSETS))  # 24 distinct lags
NL = len(LAGS)
NQ = NL // 4  # 6 quad groups of 4 lags
B, N, D, H = 4, 4096, 1024, 16
HD = 64
HH = 8           # heads per core
CW = HH * HD     # 512 channels per core
NT = 4           # channel ptiles per core (128 ch each, 2 heads per ptile)
KE = 1024        # contraction rows (biases folded into psum copies)
KP = KE // 128   # 8
PADN = 512       # zero columns in front of kT/vT for causal lookback
NTile = 512      # seq columns per ph2 tile
NNT = N // NTile  # 8
SCALE = HD ** -0.5

_CACHE = {}

# const packing offsets within cb [128, CB_W] bf16
CB_E4 = 0                  # 4 x [128, 32] selectors E_t
CB_E2 = CB_E4 + 4 * 32     # [128, 8] Z selector
CB_S2 = CB_E2 + 8          # 16 x [128, 128] broadcast selectors S_{t,s}
CB_I = CB_S2 + 16 * 128    # [128, 128] identity
CB_W = CB_I + 128


def _live(L, n0):
    # window [n0-L, n0-L+NTile) intersects valid v/k rows (reads stay inside
    # the PADN zero region otherwise)
    return L < n0 + NTile


def _build_program():
    import concourse.bacc as bacc
    import concourse.mybir as mybir
    import concourse.tile as tile

    bf16 = mybir.dt.bfloat16
    f32 = mybir.dt.float32
    Act = mybir.ActivationFunctionType

    nc = bacc.Bacc("TRN2", target_bir_lowering=False, debug=False, num_devices=8)
    xT = nc.dram_tensor("xT", [KE, N], bf16, kind="ExternalInput").ap()
    wqg = nc.dram_tensor("wqg", [KE, 4 * CW], bf16, kind="ExternalInput").ap()
    wo = nc.dram_tensor("wo", [CW, D], bf16, kind="ExternalInput").ap()
    cb = nc.dram_tensor("cb", [128, CB_W], bf16, kind="ExternalInput").ap()
    cf = nc.dram_tensor("cf", [128, NQ], f32, kind="ExternalInput").ap()
    r4 = nc.dram_tensor("r4", [HH, 4 * 128], f32, kind="ExternalInput").ap()
    bv = nc.dram_tensor("bv", [128, 16], f32, kind="ExternalInput").ap()
    yT = nc.dram_tensor("yT", [D, N], f32, kind="ExternalOutput").ap()

    with tile.TileContext(nc) as tc:
        with (
            tc.tile_pool(name="constp", bufs=1) as constp,
            tc.tile_pool(name="pers", bufs=1) as pers,
        ):
            cbt = constp.tile([128, CB_W], bf16, tag="cb")
            nc.sync.dma_start(out=cbt[:], in_=cb[:, :])
            cft = constp.tile([128, NQ], f32, tag="cf")
            nc.sync.dma_start(out=cft[:], in_=cf[:, :])
            r4t = constp.tile([HH, 4 * 128], f32, tag="r4")
            nc.sync.dma_start(out=r4t[:], in_=r4[:, :])
            bvt = constp.tile([128, 16], f32, tag="bv")
            nc.sync.dma_start(out=bvt[:], in_=bv[:, :])

            qT = [pers.tile([128, N], bf16, tag=f"qT{t}", name=f"qT{t}") for t in range(NT)]
            kT = [pers.tile([128, PADN + N], bf16, tag=f"kT{t}", name=f"kT{t}") for t in range(NT)]
            vT = [pers.tile([128, PADN + N], bf16, tag=f"vT{t}", name=f"vT{t}") for t in range(NT)]
            gT = [pers.tile([128, N], bf16, tag=f"gT{t}", name=f"gT{t}") for t in range(NT)]
            mT = qT  # qT columns are dead after stage A reads them; reuse as m
            for t in range(NT):
                nc.vector.memset(kT[t][:, 0:PADN], 0.0)
                nc.vector.memset(vT[t][:, 0:PADN], 0.0)

            # ---- Phase 1: transposed qkv+gate projection ----
            with (
                tc.tile_pool(name="xp", bufs=1) as xp,
                tc.tile_pool(name="wp", bufs=4) as wp,
                tc.tile_pool(name="pp1", bufs=8, space="PSUM") as pp1,
            ):
                for half in range(2):
                    xts = []
                    for kp in range(KP):
                        xt = xp.tile([128, N // 2], bf16, tag=f"x{kp}")
                        nc.sync.dma_start(
                            out=xt[:],
                            in_=xT[kp * 128:(kp + 1) * 128,
                                   half * (N // 2):(half + 1) * (N // 2)])
                        xts.append(xt)
                    for ch in range(16):
                        ps = [pp1.tile([128, NTile], f32, tag="p1", name="p1") for _ in range(4)]
                        for kp in range(KP):
                            wt = wp.tile([128, 128], bf16, tag="wt")
                            nc.sync.dma_start(
                                out=wt[:],
                                in_=wqg[kp * 128:(kp + 1) * 128,
                                        ch * 128:(ch + 1) * 128])
                            for ns in range(4):
                                nc.tensor.matmul(
                                    out=ps[ns][:],
                                    lhsT=wt[:],
                                    rhs=xts[kp][:, ns * NTile:(ns + 1) * NTile],
                                    start=(kp == 0), stop=(kp == KP - 1))
                        grp, t = divmod(ch, NT)
                        dst = [qT, kT, vT, gT][grp][t]
                        off = (PADN if grp in (1, 2) else 0) + half * (N // 2)
                        for ns in range(4):
                            nc.scalar.activation(
                                dst[:, off + ns * NTile: off + (ns + 1) * NTile],
                                ps[ns][:], Act.Identity, bias=bvt[:, ch:ch + 1])

            # ---- Phase 2: attention middle ----
            with (
                tc.tile_pool(name="work", bufs=4) as work,
                tc.tile_pool(name="escp", bufs=8) as escp,
                tc.tile_pool(name="rzp", bufs=2) as rzp,
                tc.tile_pool(name="scp", bufs=2, space="PSUM") as scp,
                tc.tile_pool(name="zp", bufs=1, space="PSUM") as zp,
                tc.tile_pool(name="abcp", bufs=3, space="PSUM") as abcp,
                tc.tile_pool(name="outp", bufs=2, space="PSUM") as outp,
            ):
                for nt in range(NNT):
                    n0 = nt * NTile
                    escs = []
                    zt = zp.tile([HH, NTile], f32, tag="z")
                    for q in range(NQ):
                        sct = scp.tile([128, NTile], f32, tag="sc")
                        for s in range(4):
                            L = LAGS[q * 4 + s]
                            if not _live(L, n0):
                                nc.vector.memset(sct[32 * s:32 * s + 32, :], 0.0)
                                continue
                            for t in range(NT):
                                pt = work.tile([128, NTile], bf16, tag="P")
                                nc.vector.tensor_mul(
                                    pt[:],
                                    qT[t][:, n0:n0 + NTile],
                                    kT[t][:, PADN + n0 - L: PADN + n0 - L + NTile])
                                nc.tensor.matmul(
                                    out=sct[32 * s:32 * s + 32, :],
                                    lhsT=cbt[:, CB_E4 + 32 * t: CB_E4 + 32 * (t + 1)],
                                    rhs=pt[:],
                                    start=(t == 0), stop=(t == NT - 1),
                                    tile_position=(0, 32 * s))
                        esc = escp.tile([128, NTile], bf16, tag="esc")
                        nc.scalar.activation(esc[:], sct[:], Act.Exp,
                                             scale=SCALE, bias=cft[:, q:q + 1])
                        escs.append(esc)
                        nc.tensor.matmul(
                            out=zt[:],
                            lhsT=cbt[:, CB_E2: CB_E2 + HH],
                            rhs=esc[:],
                            start=(q == 0), stop=(q == NQ - 1))
                    rz = rzp.tile([HH, NTile], f32, tag="rz")
                    nc.vector.reciprocal(rz[:], zt[:])

                    live = [(q, s) for q in range(NQ) for s in range(4)
                            if _live(LAGS[q * 4 + s], n0)]
                    gss = []
                    for t in range(NT):
                        gs = work.tile([128, NTile], bf16, tag=f"gs{t}",
                                       name=f"gs{t}")
                        nc.scalar.activation(gs[:], gT[t][:, n0:n0 + NTile],
                                             Act.Sigmoid)
                        gss.append(gs)
                    for t in range(NT):
                        outps = outp.tile([128, NTile], f32, tag="out")
                        nlive = len(live)
                        pend = []  # skew-2 software pipeline: abc runs ahead

                        def flush_one(outps=outps, pend=pend, nlive=nlive):
                            i, prod = pend.pop(0)
                            nc.tensor.matmul(
                                out=outps[:],
                                lhsT=cbt[:, CB_I: CB_I + 128],
                                rhs=prod[:],
                                start=(i == 0), stop=(i == nlive - 1))

                        for i, (q, s) in enumerate(live):
                            L = LAGS[q * 4 + s]
                            abc = abcp.tile([128, NTile], f32, tag="abc")
                            nc.tensor.matmul(
                                out=abc[:],
                                lhsT=cbt[:, CB_S2 + (t * 4 + s) * 128:
                                         CB_S2 + (t * 4 + s + 1) * 128],
                                rhs=escs[q][:],
                                start=True, stop=True)
                            absb = work.tile([128, NTile], bf16, tag="absb")
                            nc.scalar.activation(absb[:], abc[:], Act.Copy)
                            prod = work.tile([128, NTile], bf16, tag="prod")
                            nc.vector.tensor_mul(
                                prod[:], absb[:],
                                vT[t][:, PADN + n0 - L: PADN + n0 - L + NTile])
                            pend.append((i, prod))
                            if len(pend) > 2:
                                flush_one()
                        while pend:
                            flush_one()
                        # finalize: m = sigmoid(g) * (1/Z bcast) * gathered
                        rzb = abcp.tile([128, NTile], f32, tag="abc")
                        nc.tensor.matmul(
                            out=rzb[:],
                            lhsT=r4t[:, t * 128:(t + 1) * 128],
                            rhs=rz[:],
                            start=True, stop=True)
                        tmp = work.tile([128, NTile], bf16, tag="tmp")
                        nc.vector.tensor_mul(tmp[:], gss[t][:], rzb[:])
                        nc.vector.tensor_mul(
                            mT[t][:, n0:n0 + NTile], tmp[:], outps[:])

            # ---- Phase 3: transposed out projection ----
            with (
                tc.tile_pool(name="wp3", bufs=4) as wp3,
                tc.tile_pool(name="ys", bufs=4) as ys,
                tc.tile_pool(name="pp3", bufs=8, space="PSUM") as pp3,
            ):
                for do in range(D // 128):
                    pss = [pp3.tile([128, NTile], f32, tag="p3", name="p3") for _ in range(NNT)]
                    for ct in range(NT):
                        wt3 = wp3.tile([128, 128], bf16, tag="wt3")
                        nc.sync.dma_start(
                            out=wt3[:],
                            in_=wo[ct * 128:(ct + 1) * 128, do * 128:(do + 1) * 128])
                        for ns in range(NNT):
                            nc.tensor.matmul(
                                out=pss[ns][:],
                                lhsT=wt3[:],
                                rhs=mT[ct][:, ns * NTile:(ns + 1) * NTile],
                                start=(ct == 0), stop=(ct == NT - 1))
                    for ns in range(NNT):
                        yst = ys.tile([128, NTile], f32, tag="yst")
                        nc.scalar.activation(yst[:], pss[ns][:], Act.Copy)
                        nc.sync.dma_start(
                            out=yT[do * 128:(do + 1) * 128,
                                   ns * NTile:(ns + 1) * NTile],
                            in_=yst[:])

    nc.compile()
    return nc


def _get_program():
    if "nc" not in _CACHE:
        _CACHE["nc"] = _build_program()
    return _CACHE["nc"]


def _core_inputs(x, w_qkv, b_qkv, w_gate, b_gate, w_out, pos_bias, b, half):
    bf = ml_dtypes.bfloat16
    cs = slice(half * CW, (half + 1) * CW)

    xTa = x[b].T.astype(bf)

    wqg = np.empty((KE, 4 * CW), dtype=np.float32)
    wqg[:, 0:CW] = w_qkv[:, cs]
    wqg[:, CW:2 * CW] = w_qkv[:, D + cs.start:D + cs.stop]
    wqg[:, 2 * CW:3 * CW] = w_qkv[:, 2 * D + cs.start:2 * D + cs.stop]
    wqg[:, 3 * CW:4 * CW] = w_gate[:, cs]

    bcat = np.concatenate([b_qkv[cs], b_qkv[D + cs.start:D + cs.stop],
                           b_qkv[2 * D + cs.start:2 * D + cs.stop],
                           b_gate[cs]]).astype(np.float32)
    bva = bcat.reshape(16, 128).T.copy()  # bva[p, ch] = bias[ch*128 + p]

    # wl[h, j] = sum over duplicate offsets of exp(pos_bias[i, h])  (local heads)
    wl = np.zeros((HH, NL), dtype=np.float64)
    for i, off in enumerate(OFFSETS):
        j = LAGS.index(off)
        wl[:, j] += np.exp(pos_bias[i, half * HH:(half + 1) * HH].astype(np.float64))
    lnwl = np.log(wl)  # [HH, NL]

    # bf16 consts: selectors
    cba = np.zeros((128, CB_W), dtype=bf)
    p = np.arange(128)
    for t in range(NT):
        for j in range(2):  # local heads 2t, 2t+1
            col = CB_E4 + 32 * t + 2 * t + j
            cba[:, col] = (p // 64 == j).astype(bf)
    for h in range(HH):
        cba[:, CB_E2 + h] = (p % 32 == h).astype(bf)
    for t in range(NT):
        for s in range(4):
            blk = np.zeros((128, 128), dtype=bf)
            for pd in range(128):
                blk[32 * s + 2 * t + pd // 64, pd] = 1
            cba[:, CB_S2 + (t * 4 + s) * 128: CB_S2 + (t * 4 + s + 1) * 128] = blk
    cba[:, CB_I: CB_I + 128] = np.eye(128, dtype=np.float32).astype(bf)

    # f32 consts: lnwl at rows 32*s + h, col q
    cfa = np.zeros((128, NQ), dtype=np.float32)
    for q in range(NQ):
        for s in range(4):
            for h in range(HH):
                cfa[32 * s + h, q] = lnwl[h, q * 4 + s]

    r4a = np.zeros((HH, 4 * 128), dtype=np.float32)
    for t in range(NT):
        for pd in range(128):
            r4a[2 * t + pd // 64, t * 128 + pd] = 1.0

    return {
        "xT": xTa,
        "wqg": wqg.astype(bf),
        "wo": w_out[cs, :].astype(bf),
        "cb": cba,
        "cf": cfa,
        "r4": r4a,
        "bv": bva,
    }


def _in_maps(inputs):
    return [
        _core_inputs(inputs["x"], inputs["w_qkv"], inputs["b_qkv"], inputs["w_gate"],
                     inputs["b_gate"], inputs["w_out"], inputs["pos_bias"], c // 2, c % 2)
        for c in range(8)
    ]


def kernel(x, w_qkv, b_qkv, w_out, b_out, w_gate, b_gate, pos_bias):
    from concourse.bass_utils import run_bass_kernel_spmd

    x = np.asarray(x, dtype=np.float32)
    w_qkv = np.asarray(w_qkv, dtype=np.float32)
    b_qkv = np.asarray(b_qkv, dtype=np.float32)
    w_out = np.asarray(w_out, dtype=np.float32)
    b_out = np.asarray(b_out, dtype=np.float32)
    w_gate = np.asarray(w_gate, dtype=np.float32)
    b_gate = np.asarray(b_gate, dtype=np.float32)
    pos_bias = np.asarray(pos_bias, dtype=np.float32)

    nc = _get_program()
    in_maps = _in_maps({
        "x": x, "w_qkv": w_qkv, "b_qkv": b_qkv, "w_gate": w_gate,
        "b_gate": b_gate, "w_out": w_out, "pos_bias": pos_bias,
    })
    res = run_bass_kernel_spmd(nc, in_maps, core_ids=list(range(8)))
    out = np.empty((B, N, D), dtype=np.float32)
    for b in range(B):
        out[b] = (res.results[2 * b]["yT"] + res.results[2 * b + 1]["yT"]).T \
            + b_out[None, :]
    return out


# revision 9
# speedup vs baseline: 2.3589x; 1.0306x over previous
"""DSQG attention kernel for 8 Trainium2 NeuronCores — v2 (transposed layout).

Sharding: core c = (b, half): batch b = c//2, heads [half*8, half*8+8).

All on-chip tensors live in TRANSPOSED layout [channels on partitions, seq on
free]: causal lag shifts become free-dim slices (no partition-alignment issues,
no DMA window fetches). Per core:

  ph1: qT/kT/vT/gT [512ch, 4096] = W^T-stationary matmuls over xT (bias via
       ones-row). kT/vT carry 512 zero-pad columns for causal lookback.
  A:   per (ptile t, lag L): P = qT_t * kT_t[:, n-L]  (DVE, bf16 2x)
       scores via selector-matmul  sc[32s+h, n] += E_t^T @ P  (TensorE reduce
       over d; 24 lags in 6 PSUM "quad" banks, 32-aligned slots).
  B:   esc = exp(SCALE*sc + ln wl)  (ScalarE, dup-offset pos_bias weights wl
       folded via per-partition bias); Z via selector-matmul; rz = 1/Z.
  C:   abc = S_{t,s}^T @ esc  (TensorE row-broadcast) -> ScalarE copy to bf16
       -> prod = abc * vT_t[:, n-L]  (DVE 2x) -> out_t += I^T @ prod
       (TensorE identity-matmul accumulation in PSUM).
  m:   mT_t = sigmoid(gT_t) * (R4_t^T @ rz) * out_t   (normalize at the end)
  ph3: yT[Do, n] = wo^T-stationary matmuls over mT; host transposes, sums the
       two half partials and adds b_out.
"""
import sys

sys.path.insert(0, "/opt/trn_rl_repo")

import numpy as np
import ml_dtypes

N_SCALES = 11
N_TAPS = 4
OFFSETS = [(1 << j) * tau for j in range(N_SCALES) for tau in range(N_TAPS)]
LAGS = sorted(set(OFFSETS))  # 24 distinct lags
NL = len(LAGS)
NQ = NL // 4  # 6 quad groups of 4 lags
B, N, D, H = 4, 4096, 1024, 16
HD = 64
HH = 8           # heads per core
CW = HH * HD     # 512 channels per core
NT = 4           # channel ptiles per core (128 ch each, 2 heads per ptile)
KE = 1024        # contraction rows (biases folded into psum copies)
KP = KE // 128   # 8
PADN = 512       # zero columns in front of kT/vT for causal lookback
NTile = 512      # seq columns per ph2 tile
NNT = N // NTile  # 8
SCALE = HD ** -0.5

_CACHE = {}

# const packing offsets within cb [128, CB_W] bf16
CB_E4 = 0                  # 4 x [128, 32] selectors E_t
CB_E2 = CB_E4 + 4 * 32     # [128, 8] Z selector
CB_S2 = CB_E2 + 8          # 16 x [128, 128] broadcast selectors S_{t,s}
CB_I = CB_S2 + 16 * 128    # [128, 128] identity
CB_W = CB_I + 128


def _live(L, n0):
    # window [n0-L, n0-L+NTile) intersects valid v/k rows (reads stay inside
    # the PADN zero region otherwise)
    return L < n0 + NTile


def _build_program():
    import concourse.bacc as bacc
    import concourse.mybir as mybir
    import concourse.tile as tile

    bf16 = mybir.dt.bfloat16
    f32 = mybir.dt.float32
    Act = mybir.ActivationFunctionType

    nc = bacc.Bacc("TRN2", target_bir_lowering=False, debug=False, num_devices=8)
    xT = nc.dram_tensor("xT", [KE, N], bf16, kind="ExternalInput").ap()
    wqg = nc.dram_tensor("wqg", [KE, 4 * CW], bf16, kind="ExternalInput").ap()
    wo = nc.dram_tensor("wo", [CW, D], bf16, kind="ExternalInput").ap()
    cb = nc.dram_tensor("cb", [128, CB_W], bf16, kind="ExternalInput").ap()
    cf = nc.dram_tensor("cf", [128, NQ], f32, kind="ExternalInput").ap()
    r4 = nc.dram_tensor("r4", [HH, 4 * 128], f32, kind="ExternalInput").ap()
    bv = nc.dram_tensor("bv", [128, 16], f32, kind="ExternalInput").ap()
    yT = nc.dram_tensor("yT", [D, N], f32, kind="ExternalOutput").ap()

    with tile.TileContext(nc) as tc:
        with (
            tc.tile_pool(name="constp", bufs=1) as constp,
            tc.tile_pool(name="pers", bufs=1) as pers,
        ):
            cbt = constp.tile([128, CB_W], bf16, tag="cb")
            nc.sync.dma_start(out=cbt[:], in_=cb[:, :])
            cft = constp.tile([128, NQ], f32, tag="cf")
            nc.sync.dma_start(out=cft[:], in_=cf[:, :])
            r4t = constp.tile([HH, 4 * 128], f32, tag="r4")
            nc.sync.dma_start(out=r4t[:], in_=r4[:, :])
            bvt = constp.tile([128, 16], f32, tag="bv")
            nc.sync.dma_start(out=bvt[:], in_=bv[:, :])

            qT = [pers.tile([128, N], bf16, tag=f"qT{t}", name=f"qT{t}") for t in range(NT)]
            kT = [pers.tile([128, PADN + N], bf16, tag=f"kT{t}", name=f"kT{t}") for t in range(NT)]
            vT = [pers.tile([128, PADN + N], bf16, tag=f"vT{t}", name=f"vT{t}") for t in range(NT)]
            gT = [pers.tile([128, N], bf16, tag=f"gT{t}", name=f"gT{t}") for t in range(NT)]
            mT = qT  # qT columns are dead after stage A reads them; reuse as m
            for t in range(NT):
                nc.vector.memset(kT[t][:, 0:PADN], 0.0)
                nc.vector.memset(vT[t][:, 0:PADN], 0.0)

            # ---- Phase 1: transposed qkv+gate projection ----
            with (
                tc.tile_pool(name="xp", bufs=1) as xp,
                tc.tile_pool(name="wp", bufs=4) as wp,
                tc.tile_pool(name="pp1", bufs=8, space="PSUM") as pp1,
            ):
                for half in range(2):
                    xts = []
                    for kp in range(KP):
                        xt = xp.tile([128, N // 2], bf16, tag=f"x{kp}")
                        nc.sync.dma_start(
                            out=xt[:],
                            in_=xT[kp * 128:(kp + 1) * 128,
                                   half * (N // 2):(half + 1) * (N // 2)])
                        xts.append(xt)
                    for ch in range(16):
                        ps = [pp1.tile([128, NTile], f32, tag="p1", name="p1") for _ in range(4)]
                        for kp in range(KP):
                            wt = wp.tile([128, 128], bf16, tag="wt")
                            eng = nc.sync if (kp % 2 == 0) else nc.scalar
                            eng.dma_start(
                                out=wt[:],
                                in_=wqg[kp * 128:(kp + 1) * 128,
                                        ch * 128:(ch + 1) * 128])
                            for ns in range(4):
                                nc.tensor.matmul(
                                    out=ps[ns][:],
                                    lhsT=wt[:],
                                    rhs=xts[kp][:, ns * NTile:(ns + 1) * NTile],
                                    start=(kp == 0), stop=(kp == KP - 1))
                        grp, t = divmod(ch, NT)
                        dst = [qT, kT, vT, gT][grp][t]
                        off = (PADN if grp in (1, 2) else 0) + half * (N // 2)
                        for ns in range(4):
                            nc.scalar.activation(
                                dst[:, off + ns * NTile: off + (ns + 1) * NTile],
                                ps[ns][:], Act.Identity, bias=bvt[:, ch:ch + 1])

            # ---- Phase 2: attention middle ----
            with (
                tc.tile_pool(name="work", bufs=4) as work,
                tc.tile_pool(name="pwork", bufs=26) as pwork,
                tc.tile_pool(name="escp", bufs=8) as escp,
                tc.tile_pool(name="rzp", bufs=2) as rzp,
                tc.tile_pool(name="scp", bufs=2, space="PSUM") as scp,
                tc.tile_pool(name="zp", bufs=1, space="PSUM") as zp,
                tc.tile_pool(name="abcp", bufs=3, space="PSUM") as abcp,
                tc.tile_pool(name="outp", bufs=2, space="PSUM") as outp,
            ):
                def emit_pmul(nt, q, s, t, ptiles):
                    n0 = nt * NTile
                    L = LAGS[q * 4 + s]
                    pt = pwork.tile([128, NTile], bf16, tag="P", name="P")
                    nc.vector.tensor_mul(
                        pt[:],
                        qT[t][:, n0:n0 + NTile],
                        kT[t][:, PADN + n0 - L: PADN + n0 - L + NTile])
                    ptiles[(q, s, t)] = pt

                def amul_list(nt):
                    n0 = nt * NTile
                    return [(q, s, t) for q in range(NQ) for s in range(4)
                            if _live(LAGS[q * 4 + s], n0) for t in range(NT)]

                pm_ahead = {}  # P tiles pre-produced during previous C loop

                for nt in range(NNT):
                    n0 = nt * NTile
                    escs = []
                    zt = zp.tile([HH, NTile], f32, tag="z")
                    for q in range(NQ):
                        sct = scp.tile([128, NTile], f32, tag="sc")
                        for s in range(4):
                            L = LAGS[q * 4 + s]
                            if not _live(L, n0):
                                nc.vector.memset(sct[32 * s:32 * s + 32, :], 0.0)
                                continue
                            for t in range(NT):
                                if (q, s, t) not in pm_ahead:
                                    emit_pmul(nt, q, s, t, pm_ahead)
                                pt = pm_ahead.pop((q, s, t))
                                nc.tensor.matmul(
                                    out=sct[32 * s:32 * s + 32, :],
                                    lhsT=cbt[:, CB_E4 + 32 * t: CB_E4 + 32 * (t + 1)],
                                    rhs=pt[:],
                                    start=(t == 0), stop=(t == NT - 1),
                                    tile_position=(0, 32 * s))
                        esc = escp.tile([128, NTile], bf16, tag="esc")
                        nc.scalar.activation(esc[:], sct[:], Act.Exp,
                                             scale=SCALE, bias=cft[:, q:q + 1])
                        escs.append(esc)
                        nc.tensor.matmul(
                            out=zt[:],
                            lhsT=cbt[:, CB_E2: CB_E2 + HH],
                            rhs=esc[:],
                            start=(q == 0), stop=(q == NQ - 1))
                    rz = rzp.tile([HH, NTile], f32, tag="rz")
                    nc.vector.reciprocal(rz[:], zt[:])
                    # A-muls of nt+1 to interleave into this nt's C loop
                    next_am = amul_list(nt + 1) if nt + 1 < NNT else []
                    next_am = list(reversed(next_am))  # pop() from the front

                    live = [(q, s) for q in range(NQ) for s in range(4)
                            if _live(LAGS[q * 4 + s], n0)]
                    gss = []
                    for t in range(NT):
                        gs = work.tile([128, NTile], bf16, tag=f"gs{t}",
                                       name=f"gs{t}", bufs=2)
                        nc.scalar.activation(gs[:], gT[t][:, n0:n0 + NTile],
                                             Act.Sigmoid)
                        gss.append(gs)
                    for t in range(NT):
                        outps = outp.tile([128, NTile], f32, tag="out")
                        nlive = len(live)
                        pend = []  # skew-2 software pipeline: abc runs ahead

                        def flush_one(outps=outps, pend=pend, nlive=nlive):
                            i, prod = pend.pop(0)
                            nc.tensor.matmul(
                                out=outps[:],
                                lhsT=cbt[:, CB_I: CB_I + 128],
                                rhs=prod[:],
                                start=(i == 0), stop=(i == nlive - 1))

                        for i, (q, s) in enumerate(live):
                            if next_am and len(pm_ahead) < 20:
                                qq, ss, tt = next_am.pop()
                                emit_pmul(nt + 1, qq, ss, tt, pm_ahead)
                            L = LAGS[q * 4 + s]
                            abc = abcp.tile([128, NTile], f32, tag="abc")
                            nc.tensor.matmul(
                                out=abc[:],
                                lhsT=cbt[:, CB_S2 + (t * 4 + s) * 128:
                                         CB_S2 + (t * 4 + s + 1) * 128],
                                rhs=escs[q][:],
                                start=True, stop=True)
                            absb = work.tile([128, NTile], bf16, tag="absb")
                            nc.scalar.activation(absb[:], abc[:], Act.Copy)
                            prod = work.tile([128, NTile], bf16, tag="prod")
                            nc.vector.tensor_mul(
                                prod[:], absb[:],
                                vT[t][:, PADN + n0 - L: PADN + n0 - L + NTile])
                            pend.append((i, prod))
                            if len(pend) > 2:
                                flush_one()
                        while pend:
                            flush_one()
                        # finalize: m = sigmoid(g) * (1/Z bcast) * gathered
                        rzb = abcp.tile([128, NTile], f32, tag="abc")
                        nc.tensor.matmul(
                            out=rzb[:],
                            lhsT=r4t[:, t * 128:(t + 1) * 128],
                            rhs=rz[:],
                            start=True, stop=True)
                        tmp = work.tile([128, NTile], bf16, tag="tmp")
                        nc.vector.tensor_mul(tmp[:], gss[t][:], rzb[:])
                        nc.vector.tensor_mul(
                            mT[t][:, n0:n0 + NTile], tmp[:], outps[:])

            # ---- Phase 3: transposed out projection ----
            with (
                tc.tile_pool(name="wp3", bufs=4) as wp3,
                tc.tile_pool(name="ys", bufs=4) as ys,
                tc.tile_pool(name="pp3", bufs=8, space="PSUM") as pp3,
            ):
                for do in range(D // 128):
                    pss = [pp3.tile([128, NTile], f32, tag="p3", name="p3") for _ in range(NNT)]
                    for ct in range(NT):
                        wt3 = wp3.tile([128, 128], bf16, tag="wt3")
                        nc.sync.dma_start(
                            out=wt3[:],
                            in_=wo[ct * 128:(ct + 1) * 128, do * 128:(do + 1) * 128])
                        for ns in range(NNT):
                            nc.tensor.matmul(
                                out=pss[ns][:],
                                lhsT=wt3[:],
                                rhs=mT[ct][:, ns * NTile:(ns + 1) * NTile],
                                start=(ct == 0), stop=(ct == NT - 1))
                    for ns in range(NNT):
                        yst = ys.tile([128, NTile], f32, tag="yst")
                        nc.scalar.activation(yst[:], pss[ns][:], Act.Copy)
                        nc.sync.dma_start(
                            out=yT[do * 128:(do + 1) * 128,
                                   ns * NTile:(ns + 1) * NTile],
                            in_=yst[:])

    nc.compile()
    return nc


def _get_program():
    if "nc" not in _CACHE:
        _CACHE["nc"] = _build_program()
    return _CACHE["nc"]


def _core_inputs(x, w_qkv, b_qkv, w_gate, b_gate, w_out, pos_bias, b, half):
    bf = ml_dtypes.bfloat16
    cs = slice(half * CW, (half + 1) * CW)

    xTa = x[b].T.astype(bf)

    wqg = np.empty((KE, 4 * CW), dtype=np.float32)
    wqg[:, 0:CW] = w_qkv[:, cs]
    wqg[:, CW:2 * CW] = w_qkv[:, D + cs.start:D + cs.stop]
    wqg[:, 2 * CW:3 * CW] = w_qkv[:, 2 * D + cs.start:2 * D + cs.stop]
    wqg[:, 3 * CW:4 * CW] = w_gate[:, cs]

    bcat = np.concatenate([b_qkv[cs], b_qkv[D + cs.start:D + cs.stop],
                           b_qkv[2 * D + cs.start:2 * D + cs.stop],
                           b_gate[cs]]).astype(np.float32)
    bva = bcat.reshape(16, 128).T.copy()  # bva[p, ch] = bias[ch*128 + p]

    # wl[h, j] = sum over duplicate offsets of exp(pos_bias[i, h])  (local heads)
    wl = np.zeros((HH, NL), dtype=np.float64)
    for i, off in enumerate(OFFSETS):
        j = LAGS.index(off)
        wl[:, j] += np.exp(pos_bias[i, half * HH:(half + 1) * HH].astype(np.float64))
    lnwl = np.log(wl)  # [HH, NL]

    # bf16 consts: selectors
    cba = np.zeros((128, CB_W), dtype=bf)
    p = np.arange(128)
    for t in range(NT):
        for j in range(2):  # local heads 2t, 2t+1
            col = CB_E4 + 32 * t + 2 * t + j
            cba[:, col] = (p // 64 == j).astype(bf)
    for h in range(HH):
        cba[:, CB_E2 + h] = (p % 32 == h).astype(bf)
    for t in range(NT):
        for s in range(4):
            blk = np.zeros((128, 128), dtype=bf)
            for pd in range(128):
                blk[32 * s + 2 * t + pd // 64, pd] = 1
            cba[:, CB_S2 + (t * 4 + s) * 128: CB_S2 + (t * 4 + s + 1) * 128] = blk
    cba[:, CB_I: CB_I + 128] = np.eye(128, dtype=np.float32).astype(bf)

    # f32 consts: lnwl at rows 32*s + h, col q
    cfa = np.zeros((128, NQ), dtype=np.float32)
    for q in range(NQ):
        for s in range(4):
            for h in range(HH):
                cfa[32 * s + h, q] = lnwl[h, q * 4 + s]

    r4a = np.zeros((HH, 4 * 128), dtype=np.float32)
    for t in range(NT):
        for pd in range(128):
            r4a[2 * t + pd // 64, t * 128 + pd] = 1.0

    return {
        "xT": xTa,
        "wqg": wqg.astype(bf),
        "wo": w_out[cs, :].astype(bf),
        "cb": cba,
        "cf": cfa,
        "r4": r4a,
        "bv": bva,
    }


def _in_maps(inputs):
    return [
        _core_inputs(inputs["x"], inputs["w_qkv"], inputs["b_qkv"], inputs["w_gate"],
                     inputs["b_gate"], inputs["w_out"], inputs["pos_bias"], c // 2, c % 2)
        for c in range(8)
    ]


def kernel(x, w_qkv, b_qkv, w_out, b_out, w_gate, b_gate, pos_bias):
    from concourse.bass_utils import run_bass_kernel_spmd

    x = np.asarray(x, dtype=np.float32)
    w_qkv = np.asarray(w_qkv, dtype=np.float32)
    b_qkv = np.asarray(b_qkv, dtype=np.float32)
    w_out = np.asarray(w_out, dtype=np.float32)
    b_out = np.asarray(b_out, dtype=np.float32)
    w_gate = np.asarray(w_gate, dtype=np.float32)
    b_gate = np.asarray(b_gate, dtype=np.float32)
    pos_bias = np.asarray(pos_bias, dtype=np.float32)

    nc = _get_program()
    in_maps = _in_maps({
        "x": x, "w_qkv": w_qkv, "b_qkv": b_qkv, "w_gate": w_gate,
        "b_gate": b_gate, "w_out": w_out, "pos_bias": pos_bias,
    })
    res = run_bass_kernel_spmd(nc, in_maps, core_ids=list(range(8)))
    out = np.empty((B, N, D), dtype=np.float32)
    for b in range(B):
        out[b] = (res.results[2 * b]["yT"] + res.results[2 * b + 1]["yT"]).T \
            + b_out[None, :]
    return out
